# revision 18
# baseline (speedup 1.0000x reference)
"""DeepseekV3 decoder layer (MLA attention + dense MLP) on 8 trn2 NeuronCores.

v3: fp8 (e4m3) DoubleRow tensor-parallel kernel.

Changes vs v2 (AllGather-everywhere bf16):
- All attention-side GEMMs (q_a/kv_a, q_b/kv_b, v, probs@v, o_proj) run in
  fp8e4 with DoubleRow perf mode: 2 k-tiles per PE pass = 2x matmul
  throughput.  MLP GEMMs stay bf16 (fp8 error there lands 1:1 on the output
  and blows the 2e-2 budget; measured DoubleRow is 2x, so compensated
  schemes lose).
- o_proj flipped from AllGather(attn)+own-cols GEMM to own-heads partial
  GEMM + AllReduce: removes the attn AllGather round trip; the full h2 is
  rebuilt on every core (needed by the MLP anyway) as hT(bf16) + o_ar.
- down_proj flipped from AllGather(m)+own-cols GEMM to own-rows partial
  GEMM + ReduceScatter: m (22.5MB gathered before) never leaves SBUF.  The
  o partial is folded into the RS input, so RS directly yields
  (o + mlp)_own and the fp32 residual is added from the per-core h_ownD
  input (SPMD-safe: rank-dependence only through collectives / input data).
- probs quantize to fp8 straight out of Exp (max exp(SCALE*s) ~ 74 < 240);
  softmax denominator uses the same quantized probs so no rescaling needed.
- lq/lkv AllGathers carry fp8 (half the bytes).
"""
import sys

sys.path.insert(0, '/opt/trn_rl_repo')

import numpy as np
import ml_dtypes

S, D, H, QLORA, KVLORA = 1024, 4096, 32, 1536, 512
DN, DR, DV, INTER = 128, 64, 128, 11008
EPS = 1e-6
SCALE = (DN + DR) ** -0.5
NC = 8
HPC = H // NC               # 4 heads per core
QAC = QLORA // NC           # 192 q_a cols per core
KVAC = (KVLORA + DR) // NC  # 72 kv_a cols per core
KVAP = 80                   # padded to 16B stride for DoubleRow ldweights
OC = D // NC                # 512 output rows per core
IC = INTER // NC            # 1376 gate/up cols per core

P = 128
TCH = 512                   # token chunk
NCH = S // TCH              # 2 chunks
NDT = D // P                # 32
NKVT = KVLORA // P          # 4
NQLT = QLORA // P           # 12
NTT = S // P                # 8
NQB = HPC * (DN + DR) // P  # 6 qT row chunks (4 nope + 2 pe)
NOB = OC // P               # 4
NMC = (IC + P - 1) // P     # 11 gate/up row tiles (last is 96)
BF16 = ml_dtypes.bfloat16
F8 = ml_dtypes.float8_e4m3

# fp8 scales
SH = 16.0     # hT8 = fp8(SH * h)
SW = 256.0    # fp8 weights = fp8(SW * w)
SLQ = 16.0    # lq/lkv written as fp8(SLQ * lq_raw)
SV = 16.0     # v_sb = fp8(SV * v_true); attn then = SV * attn_true
S2 = SW * SLQ  # 4096: psum scale of fp8 b-projections

_CACHE = {}


def _build():
    import concourse.bass as bass
    import concourse.tile as tile
    from concourse import bacc, mybir
    from contextlib import ExitStack

    dt = mybir.dt
    f32, bf16, f8 = dt.float32, dt.bfloat16, dt.float8e4
    AF = mybir.ActivationFunctionType
    DR_MODE = mybir.MatmulPerfMode.DoubleRow
    ts, ds = bass.ts, bass.ds

    nc = bacc.Bacc('TRN2', target_bir_lowering=False, debug=False,
                   num_devices=NC)

    hT8 = nc.dram_tensor('hT8', [P, NCH, NDT, TCH], f8, kind='ExternalInput')
    hTb = nc.dram_tensor('hTb', [P, NCH, NDT, TCH], bf16, kind='ExternalInput')
    h_ownD = nc.dram_tensor('h_ownD', [OC, S], f32, kind='ExternalInput')
    qa_own = nc.dram_tensor('qa_own', [P, NDT, QAC], f8, kind='ExternalInput')
    kva_own = nc.dram_tensor('kva_own', [P, NDT, KVAP], f8, kind='ExternalInput')
    qb_own = nc.dram_tensor('qb_own', [NQB * P, NQLT * P], f8, kind='ExternalInput')
    kvb_own = nc.dram_tensor('kvb_own', [P, NKVT, HPC * (DN + DV)], f8,
                             kind='ExternalInput')
    o_own = nc.dram_tensor('o_own', [P, HPC, D], f8, kind='ExternalInput')
    gate_own = nc.dram_tensor('gate_own', [P, NDT * IC], bf16, kind='ExternalInput')
    up_own = nc.dram_tensor('up_own', [P, NDT * IC], bf16, kind='ExternalInput')
    down_own = nc.dram_tensor('down_own', [P, NDT * NMC * P], bf16,
                              kind='ExternalInput')
    cosT_d = nc.dram_tensor('cosT2', [P, S], bf16, kind='ExternalInput')
    sinT_d = nc.dram_tensor('sinT2', [P, S], bf16, kind='ExternalInput')
    rot2_d = nc.dram_tensor('rot2T', [P, P], bf16, kind='ExternalInput')
    masks_d = nc.dram_tensor('masks', [4, P, TCH], f8, kind='ExternalInput')
    out = nc.dram_tensor('out', [OC, S], f32, kind='ExternalOutput')

    RG = [list(range(NC))]

    def mm(psum, lhsT, rhs, start, stop):
        nc.tensor.matmul(psum, lhsT, rhs, start=start, stop=stop)

    def mm2(psum, lhsT, rhs, start, stop):
        nc.tensor.matmul(psum, lhsT, rhs, start=start, stop=stop,
                         perf_mode=DR_MODE)

    def cc(kind, in_t, out_t, op=None):
        op = op or (mybir.AluOpType.bypass if kind == 'AllGather'
                    else mybir.AluOpType.add)
        nc.gpsimd.collective_compute(
            kind, op, replica_groups=RG, ins=[in_t[:]], outs=[out_t[:]])

    with tile.TileContext(nc) as tc, ExitStack() as st:
        const = st.enter_context(tc.tile_pool(name='const', bufs=1))
        vecs = st.enter_context(tc.tile_pool(name='vecs', bufs=1))
        dram = st.enter_context(tc.tile_pool(name='dram', bufs=1, space='DRAM'))

        ones_bf = const.tile([P, 1], bf16)
        nc.vector.memset(ones_bf, 1.0)
        ones8w = const.tile([P, 2, 16], f8)
        nc.vector.memset(ones8w, 1.0)
        ones8 = ones8w[:, :, 0:1]
        onesrow_bf = const.tile([1, P], bf16)
        nc.vector.memset(onesrow_bf, 1.0)
        eps1 = const.tile([1, 1], f32)
        nc.vector.memset(eps1, EPS)
        epsq = const.tile([1, 1], f32)
        nc.vector.memset(epsq, EPS * S2 * S2)

        lq_dram = [dram.tile([QAC, TCH], f8, name=f'lq_dram{c}')
                   for c in range(NCH)]
        lq_ag = [dram.tile([QLORA, TCH], f8, addr_space='Shared',
                           name=f'lq_ag{c}') for c in range(NCH)]
        lkv_dram = [dram.tile([KVAC, TCH], f8, name=f'lkv_dram{c}')
                    for c in range(NCH)]
        lkv_ag = [dram.tile([KVLORA + DR, TCH], f8, addr_space='Shared',
                            name=f'lkv_ag{c}') for c in range(NCH)]
        opart_dram = [dram.tile([D, TCH], bf16, name=f'opart{c}')
                      for c in range(NCH)]
        oar_dram = [dram.tile([D, TCH], bf16, addr_space='Shared',
                              name=f'oar{c}') for c in range(NCH)]
        pd_dram = [dram.tile([NC * 2 * P, S], bf16, name=f'pd{h}')
                   for h in range(2)]
        rs_dram = [dram.tile([2 * P, S], bf16, name=f'rs{h}')
                   for h in range(2)]

        # ---- helpers ----------------------------------------------------
        def vrow(name):
            return vecs.tile([1, TCH], f32, tag='vrow', bufs=4, name=name)

        def bcast_row(row_fp32, name, pool, ps_pool, ps_bufs=1, bufs=1):
            """[1,TCH] fp32 -> [P,TCH] fp32 SBUF (bf16 precision) via matmul."""
            rb = pool.tile([1, TCH], bf16, tag='brow', bufs=3, name=f'{name}_r')
            nc.vector.tensor_copy(rb, row_fp32)
            ps = ps_pool.tile([P, TCH], f32, tag='bc_ps', bufs=ps_bufs,
                              name=f'{name}_ps')
            mm(ps, onesrow_bf, rb[0:1, :], True, True)
            sb = pool.tile([P, TCH], f32, tag=f'{name}_bc', bufs=bufs,
                           name=f'{name}_bc')
            nc.vector.tensor_copy(sb, ps)
            return sb

        def finish_norm(ps_sum, scale_meanN, name, extra_sq=None, tag='vrow',
                        bias=None):
            sb = vecs.tile([1, TCH], f32, tag=tag, bufs=4, name=name)
            if extra_sq is not None:
                nc.vector.tensor_mul(sb, ps_sum, extra_sq)
            else:
                nc.vector.tensor_copy(sb, ps_sum)
            nc.scalar.activation(sb, sb, AF.Sqrt, bias=bias if bias is not None
                                 else eps1, scale=scale_meanN)
            nc.vector.reciprocal_approx_fast(out=sb, in_=sb)
            return sb

        def sq_chains(get_src, n, width, pool, tag, nacc, k_lo=0, k_hi=None,
                      accs=None):
            """acc[a] accumulates get_src(k)^2 (ACT square + DVE adds)."""
            if k_hi is None:
                k_hi = n
            if accs is None:
                accs = [pool.tile([P, width], f32, tag=f'{tag}a{a}', bufs=1,
                                  name=f'{tag}a{a}') for a in range(nacc)]
            for k in range(k_lo, k_hi):
                a = k % nacc
                if k < nacc:
                    nc.scalar.activation(accs[a], get_src(k), AF.Square)
                else:
                    sq = pool.tile([P, width], f32, tag=f'{tag}s', bufs=2,
                                   name=f'{tag}s')
                    nc.scalar.activation(sq, get_src(k), AF.Square)
                    nc.vector.tensor_add(accs[a], accs[a], sq)
            return accs

        def sq_reduce(accs, cs, pool, ps_pool, tag, ps_bufs=1):
            ps = ps_pool.tile([1, TCH], f32, tag=tag, bufs=ps_bufs, name=tag)
            for a, acc in enumerate(accs):
                ab = pool.tile([P, TCH], bf16, tag='accb', bufs=2, name='accb')
                nc.vector.tensor_copy(ab, acc[:, cs] if cs is not None else acc)
                mm(ps, ones_bf, ab, a == 0, a == len(accs) - 1)
            return ps

        # warmup collective: pays the first-CC barrier/ramp cost early
        warm_in = dram.tile([P, 16], f8, name='warm_in')
        warm_out = dram.tile([NC * P, 16], f8, addr_space='Shared',
                             name='warm_out')
        warm_sb = const.tile([P, 16], f8)
        nc.vector.memset(warm_sb, 0.0)
        nc.sync.dma_start(out=warm_in[:], in_=warm_sb)
        cc('AllGather', warm_in, warm_out)

        # ---- persistent SBUF --------------------------------------------
        r1_c = [None, None]
        cosr1_c, sinr1_c = [None, None], [None, None]
        r2_b_c = [None, None]

        mlp_sb = st.enter_context(tc.tile_pool(name='mlp_sb', bufs=1))
        h2T = mlp_sb.tile([P, NDT, S], bf16, name='h2T')

        with ExitStack() as att_scope:
            attp = att_scope.enter_context(tc.tile_pool(name='attp', bufs=1))
            qT = attp.tile([P, NQB, S], bf16, name='qT')
            kT = attp.tile([P, HPC, S], bf16, name='kT')
            v_sb = attp.tile([P, NTT, HPC * DV], f8, name='v_sb')
            kpe = attp.tile([P, S], bf16, name='kpe')
            cos_sb = attp.tile([P, S], bf16, name='cos_sb')
            nc.sync.dma_start(out=cos_sb, in_=cosT_d[:])
            sin_sb = attp.tile([P, S], bf16, name='sin_sb')
            nc.sync.dma_start(out=sin_sb, in_=sinT_d[:])
            rot2_sb = attp.tile([P, P], bf16, name='rot2_sb')
            nc.sync.dma_start(out=rot2_sb, in_=rot2_d[:])
            masks_sb = attp.tile([P, 4, TCH], f8, name='masks_sb')
            nc.sync.dma_start(out=masks_sb, in_=masks_d.rearrange('m p c -> p m c'))
            qa_sb = attp.tile([P, NDT, QAC], f8, name='qa_sb')
            nc.sync.dma_start(out=qa_sb, in_=qa_own[:])
            kva_sb = attp.tile([P, NDT, KVAP], f8, name='kva_sb')
            nc.sync.dma_start(out=kva_sb, in_=kva_own[:])
            kvb_sb = attp.tile([P, NKVT, HPC * (DN + DV)], f8, name='kvb_sb')
            nc.sync.dma_start(out=kvb_sb, in_=kvb_own[:])
            o_sb = attp.tile([P, HPC, D], f8, name='o_sb')
            nc.sync.dma_start(out=o_sb, in_=o_own[:])

            # ============ phase 1: a-projections + input-norm stats =======
            with ExitStack() as ph1_scope:
                hkp = ph1_scope.enter_context(tc.tile_pool(name='hkp', bufs=1))
                ph1 = ph1_scope.enter_context(tc.tile_pool(name='ph1', bufs=1))
                ph1ps = ph1_scope.enter_context(
                    tc.tile_pool(name='ph1ps', bufs=1, space='PSUM'))
                G1 = 8
                ss_acc_c = [None, None]
                for c in range(NCH):
                    hk = hkp.tile([P, NDT, TCH], f8, tag='hk', bufs=1,
                                  name='hk')
                    for g in range(NDT // G1):
                        nc.sync.dma_start(
                            out=hk[:, g * G1:(g + 1) * G1, :],
                            in_=hT8[:, c, g * G1:(g + 1) * G1, :])
                    ps1 = ph1ps.tile([P, TCH], f32, tag='lq1', bufs=2, name='lq1')
                    ps2 = ph1ps.tile([QAC - P, TCH], f32, tag='lq2', bufs=1,
                                     name='lq2')
                    for k in range(0, NDT, 2):
                        mm2(ps1, qa_sb[:, k:k + 2, 0:P], hk[:, k:k + 2, :],
                            k == 0, k == NDT - 2)
                        mm2(ps2, qa_sb[:, k:k + 2, P:QAC], hk[:, k:k + 2, :],
                            k == 0, k == NDT - 2)
                    lq1 = ph1.tile([P, TCH], f8, tag='lq1s', bufs=2, name='lq1s')
                    nc.vector.tensor_scalar_mul(lq1, ps1, SLQ / (SH * SW))
                    nc.sync.dma_start(out=lq_dram[c][0:P, :], in_=lq1)
                    lq2 = ph1.tile([QAC - P, TCH], f8, tag='lq2s', bufs=2,
                                   name='lq2s')
                    nc.vector.tensor_scalar_mul(lq2, ps2, SLQ / (SH * SW))
                    nc.sync.dma_start(out=lq_dram[c][P:QAC, :], in_=lq2)
                    cc('AllGather', lq_dram[c], lq_ag[c])
                    pskv = ph1ps.tile([KVAP, TCH], f32, tag='lkv', bufs=1,
                                      name='lkv')
                    for k in range(0, NDT, 2):
                        mm2(pskv, kva_sb[:, k:k + 2, :], hk[:, k:k + 2, :],
                            k == 0, k == NDT - 2)
                    lkv1 = ph1.tile([KVAC, TCH], f8, tag='lkvs', bufs=2,
                                    name='lkvs')
                    nc.vector.tensor_scalar_mul(lkv1, pskv[0:KVAC, :], SLQ / (SH * SW))
                    nc.sync.dma_start(out=lkv_dram[c][:], in_=lkv1)
                    cc('AllGather', lkv_dram[c], lkv_ag[c])
                    ss_acc_c[c] = sq_chains(
                        lambda k: hk[:, k, :], NDT, TCH, ph1, f'ss{c}', 2)
                # input-norm factors per chunk (ss holds SH^2 * h^2 sums)
                for c in range(NCH):
                    cs = ts(c, TCH)
                    ss = sq_reduce(ss_acc_c[c], None, ph1,
                                   ph1ps, 'ss_ps', ps_bufs=2)
                    r1 = finish_norm(ss, 1.0 / (D * SH * SH), f'r1_{c}',
                                     tag='r1')
                    r1sq = vecs.tile([1, TCH], f32, tag='r1sq', bufs=2,
                                     name='r1sq')
                    nc.vector.tensor_mul(r1sq, r1, r1)
                    r1_c[c] = (r1, r1sq)
                    # kpe factors need r1/SLQ (lkv_ag is fp8 = SLQ*lkv_raw)
                    r1d = vecs.tile([1, TCH], f32, tag='r1d', bufs=2,
                                    name='r1d')
                    nc.vector.tensor_scalar_mul(r1d, r1, 1.0 / SLQ)
                    r1b = bcast_row(r1d, f'r1_{c}', ph1, ph1ps, ps_bufs=2,
                                    bufs=1)
                    cr = attp.tile([P, TCH], bf16, tag='cosr1', bufs=2,
                                   name='cosr1')
                    nc.vector.tensor_mul(cr, cos_sb[:, cs], r1b)
                    sr = attp.tile([P, TCH], bf16, tag='sinr1', bufs=2,
                                   name='sinr1')
                    nc.vector.tensor_mul(sr, sin_sb[:, cs], r1b)
                    cosr1_c[c], sinr1_c[c] = cr, sr

            # ============ phases 2-5 per chunk ============================
            pre = att_scope.enter_context(tc.tile_pool(name='pre', bufs=1))
            for c in range(NCH):
                cs = ts(c, TCH)
                r1, r1sq = r1_c[c]
                with tc.tile_pool(name='ph2', bufs=1) as ph2, \
                     tc.tile_pool(name='ph2w', bufs=3) as ph2w, \
                     tc.tile_pool(name='ph2ps', bufs=1, space='PSUM') as ph2ps:
                    # prefetch this chunk's residual rows into h2T early;
                    # the o_ar add happens lazily before the MLP needs it
                    nc.sync.dma_start(out=h2T[:, :, cs], in_=hTb[:, c, :, :])
                    lqn = pre.tile([P, NQLT, TCH], f8, tag='lqn', bufs=1,
                                   name='lqn')
                    for g in range(2):
                        nc.sync.dma_start(
                            out=lqn[:, g * 6:(g + 1) * 6, :],
                            in_=lq_ag[c].rearrange('(k p) s -> p k s', p=P)
                            [:, g * 6:(g + 1) * 6, :])
                    kvn = pre.tile([P, NKVT, TCH], f8, tag='kvn', bufs=1,
                                   name='kvn')
                    nc.sync.dma_start(
                        out=kvn, in_=lkv_ag[c][0:KVLORA, :]
                        .rearrange('(k p) s -> p k s', p=P))

                    # q_b GEMM mc 0-2 (PSUM qb_ps: 3 banks)
                    def qbw_tile(mc2):
                        w = ph2w.tile([P, NQLT, P], f8, tag='qbw', bufs=6,
                                      name='qbw')
                        nc.sync.dma_start(
                            out=w, in_=qb_own[ds(mc2 * P, P), :]
                            .rearrange('p (k n) -> p k n', n=P))
                        return w

                    ps_q = []
                    for mc2 in range(3):
                        w = qbw_tile(mc2)
                        ps = ph2ps.tile([P, TCH], f32, tag='qb_ps', bufs=3,
                                        name='qb_ps')
                        for k in range(0, NQLT, 2):
                            mm2(ps, w[:, k:k + 2, :], lqn[:, k:k + 2, :],
                                k == 0, k == NQLT - 2)
                        ps_q.append(ps)
                    # rq' = 1/(S2*sqrt(mean(lq^2)+eps)); fq = rq'*r1
                    acc_q = sq_chains(lambda k: lqn[:, k, :], NQLT, TCH, ph2,
                                      'st2', 2)
                    ssq = sq_reduce(acc_q, None, ph2, ph2ps, 'st_ps')
                    rq = finish_norm(ssq, (S2 * S2) / (QLORA * SLQ * SLQ),
                                     f'rq_{c}', extra_sq=r1sq, bias=epsq)
                    fq = vrow(f'fq_{c}')
                    nc.vector.tensor_mul(fq, rq, r1)
                    fq_b = bcast_row(fq, f'fq_{c}', ph2, ph2ps)
                    cf = ph2.tile([P, TCH], bf16, tag='cosfq', bufs=1,
                                  name='cosfq')
                    nc.vector.tensor_mul(cf, cos_sb[:, cs], fq_b)
                    sf = ph2.tile([P, TCH], bf16, tag='sinfq', bufs=1,
                                  name='sinfq')
                    nc.vector.tensor_mul(sf, sin_sb[:, cs], fq_b)
                    for mc2 in range(3):
                        nc.vector.tensor_mul(qT[:, mc2, cs], ps_q[mc2], fq_b)
                    # remaining q_b tiles: mc 3 (nope) + 4,5 (pe with rope)
                    for mc2 in range(3, NQB):
                        w = qbw_tile(mc2)
                        ps = ph2ps.tile([P, TCH], f32, tag='qb_ps', bufs=3,
                                        name='qb_ps')
                        for k in range(0, NQLT, 2):
                            mm2(ps, w[:, k:k + 2, :], lqn[:, k:k + 2, :],
                                k == 0, k == NQLT - 2)
                        if mc2 == 3:
                            nc.vector.tensor_mul(qT[:, mc2, cs], ps, fq_b)
                        else:
                            qraw = ph2.tile([P, TCH], bf16, tag='qraw', bufs=1,
                                            name='qraw')
                            nc.vector.tensor_copy(qraw, ps)
                            ps2 = ph2ps.tile([P, TCH], f32, tag='qrot', bufs=1,
                                             name='qrot')
                            nc.tensor.matmul(ps2, rot2_sb, qraw,
                                             start=True, stop=True)
                            rot_s = ph2.tile([P, TCH], f32, tag='rot_qs',
                                             bufs=1, name='rot_qs')
                            nc.vector.tensor_mul(rot_s, ps2, sf)
                            nc.vector.tensor_mul(qT[:, mc2, cs], qraw, cf)
                            nc.vector.tensor_add(qT[:, mc2, cs],
                                                 qT[:, mc2, cs], rot_s)

                    # kv stats: rkv true (for requant) + /S2 variant (kT)
                    acc_kv = sq_chains(lambda k: kvn[:, k, :], NKVT, TCH, ph2,
                                       'st2', 2)
                    sskv = sq_reduce(acc_kv, None, ph2, ph2ps, 'st_ps')
                    rkv = finish_norm(sskv, 1.0 / (KVLORA * SLQ * SLQ),
                                      f'rkv_{c}', extra_sq=r1sq)
                    fkvv = vrow(f'fkvv_{c}')
                    nc.vector.tensor_mul(fkvv, rkv, r1)
                    fkv = vrow(f'fkv_{c}')
                    nc.vector.tensor_scalar_mul(fkv, fkvv, 1.0 / S2)
                    fkv_b = bcast_row(fkv, f'fkv_{c}', ph2, ph2ps)
                    # kT on raw kvn, drain-scaled (PSUM kv_ps: 2 banks)
                    for j in range(HPC):
                        ps = ph2ps.tile([P, TCH], f32, tag='kv_ps', bufs=2,
                                        name='kv_ps')
                        for k in range(0, NKVT, 2):
                            mm2(ps, kvb_sb[:, k:k + 2, ts(j, DN)],
                                kvn[:, k:k + 2, :], k == 0, k == NKVT - 2)
                        nc.vector.tensor_mul(kT[:, j, cs], ps, fkv_b)
                    # requantize kvn with the norm factors for the v GEMM
                    fkvv_b = bcast_row(fkvv, f'fkvv_{c}', ph2, ph2ps)
                    kvs = ph2.tile([P, NKVT, TCH], f8, tag='kvs', bufs=1,
                                   name='kvs')
                    for k in range(NKVT):
                        nc.vector.tensor_mul(kvs[:, k, :], kvn[:, k, :], fkvv_b)
                    for i in range(4 * c, 4 * c + 4):
                        il = i - 4 * c
                        ps = ph2ps.tile([P, HPC * DV], f32, tag='kv_ps', bufs=2,
                                        name='kv_ps')
                        for k in range(0, NKVT, 2):
                            mm2(ps, kvs[:, k:k + 2, ts(il, P)],
                                kvb_sb[:, k:k + 2, HPC * DN:],
                                k == 0, k == NKVT - 2)
                        nc.vector.tensor_scalar_mul(v_sb[:, i, :], ps, SV / S2)
                    # k_pe rope: kpe = raw*(cos*r1/SLQ) + rot(raw)*(sin*r1/SLQ)
                    kpe_raw8 = ph2.tile([DR, TCH], f8, tag='kpe_raw8', bufs=1,
                                        name='kpe_raw8')
                    nc.sync.dma_start(out=kpe_raw8,
                                      in_=lkv_ag[c][KVLORA:KVLORA + DR, :])
                    kpe_raw = ph2.tile([DR, TCH], bf16, tag='kpe_raw', bufs=1,
                                       name='kpe_raw')
                    nc.vector.tensor_copy(kpe_raw, kpe_raw8)
                    ps_rot = ph2ps.tile([P, TCH], f32, tag='qrot', bufs=1,
                                        name='kperot')
                    nc.tensor.matmul(ps_rot[0:DR, :], rot2_sb[0:DR, 0:DR],
                                     kpe_raw, start=True, stop=True)
                    rot_s = ph2.tile([DR, TCH], f32, tag='kpe_rs', bufs=1,
                                     name='kpe_rs')
                    nc.vector.tensor_mul(rot_s, ps_rot[0:DR, :],
                                         sinr1_c[c][0:DR, :])
                    kpe_t = ph2.tile([DR, TCH], f32, tag='kpe_t', bufs=1,
                                     name='kpe_t')
                    nc.vector.tensor_mul(kpe_t, kpe_raw, cosr1_c[c][0:DR, :])
                    nc.vector.tensor_add(kpe[0:DR, cs], kpe_t, rot_s)
                    nc.sync.dma_start(out=kpe[DR:P, cs], in_=kpe[0:DR, cs])

                # ---- attention for this chunk ----------------------------
                # PSUM: sc 2 + se 2 + at 2 + bc 2 = 8 banks
                attn_loc = None
                with tc.tile_pool(name='ph4', bufs=1) as ph4, \
                     tc.tile_pool(name='ph4p', bufs=1) as ph4p, \
                     tc.tile_pool(name='ph4ps', bufs=1, space='PSUM') as ph4ps:
                    attn_loc = pre.tile([P, HPC, TCH], f8, tag='attn_loc',
                                        bufs=2, name='attn_loc')
                    ilist = list(range(4 * c + 4))
                    npair = len(ilist) // 2
                    for j in range(HPC):
                        pe_mc = HPC * DN // P + (j * DR) // P
                        pe_off = (j * DR) % P
                        epairs = []
                        for n in range(npair):
                            ep = ph4p.tile([P, 2, TCH], f8, tag=f'probs{n}',
                                           bufs=2, name=f'probs{n}')
                            for half in range(2):
                                i = 2 * n + half
                                ps = ph4ps.tile([P, TCH], f32, tag='sc_ps',
                                                bufs=2, name='sc_ps')
                                mm(ps, kT[:, j, ts(i, P)], qT[:, j, cs],
                                   True, False)
                                mm(ps, kpe[pe_off:pe_off + DR, ts(i, P)],
                                   qT[pe_off:pe_off + DR, pe_mc, cs],
                                   False, True)
                                nc.scalar.activation(ep[:, half, :], ps, AF.Exp,
                                                     scale=SCALE)
                                if i // 4 == c:
                                    nc.vector.tensor_mul(
                                        ep[:, half, :], ep[:, half, :],
                                        masks_sb[:, i % 4, :])
                            epairs.append(ep)
                        ps_se = ph4ps.tile([1, TCH], f32, tag='se_ps', bufs=2,
                                           name='se_ps')
                        for n, ep in enumerate(epairs):
                            mm2(ps_se, ones8, ep, n == 0, n == npair - 1)
                        ps_at = ph4ps.tile([P, TCH], f32, tag='at_ps', bufs=2,
                                           name='at_ps')
                        for n, ep in enumerate(epairs):
                            mm2(ps_at, v_sb[:, 2 * n:2 * n + 2, ts(j, DV)], ep,
                                n == 0, n == npair - 1)
                        recip = vrow(f'recip_{c}_{j}')
                        sef = vrow(f'se_{c}_{j}')
                        nc.vector.tensor_copy(sef, ps_se)
                        nc.vector.reciprocal_approx_fast(out=recip, in_=sef)
                        recip_b = bcast_row(recip, 'recip', ph4, ph4ps,
                                            ps_bufs=2, bufs=1)
                        nc.vector.tensor_mul(attn_loc[:, j, :], ps_at, recip_b)

                # ---- o_proj partial (own heads) + AllReduce --------------
                with tc.tile_pool(name='ph5', bufs=1) as ph5, \
                     tc.tile_pool(name='ph5ps', bufs=1, space='PSUM') as ph5ps:
                    for g in range(NDT):
                        ps = ph5ps.tile([P, TCH], f32, tag='o_ps', bufs=4,
                                        name='o_ps')
                        mm2(ps, o_sb[:, 0:2, ts(g, P)], attn_loc[:, 0:2, :],
                            True, False)
                        mm2(ps, o_sb[:, 2:4, ts(g, P)], attn_loc[:, 2:4, :],
                            False, True)
                        ot = ph5.tile([P, TCH], bf16, tag='ot', bufs=4,
                                      name='ot')
                        nc.vector.tensor_scalar_mul(ot, ps, 1.0 / (SW * SV))
                        nc.sync.dma_start(out=opart_dram[c][ts(g, P), :],
                                          in_=ot)
                cc('AllReduce', opart_dram[c], oar_dram[c])

        # ============ phase 6: post-norm stats + gate/up (lag pipeline) ====
        msp = st.enter_context(tc.tile_pool(name='msp', bufs=1))
        m_sb = msp.tile([P, NMC, S], bf16, name='m_sb')
        nc.vector.memset(m_sb[96:P, NMC - 1, :], 0.0)
        with ExitStack() as mlp_scope:
            ph6 = mlp_scope.enter_context(tc.tile_pool(name='ph6', bufs=1))
            ph6w = mlp_scope.enter_context(tc.tile_pool(name='ph6w', bufs=1))
            ph6ps = mlp_scope.enter_context(
                tc.tile_pool(name='ph6ps', bufs=1, space='PSUM'))

            def build_h2(c):
                # h2T[:, :, cs] += o_ar (residual rows were DMA'd in ph2)
                cs = ts(c, TCH)
                for g in range(NDT // 8):
                    oar_sb = ph6.tile([P, 8, TCH], bf16, tag='oar_sb', bufs=2,
                                      name='oar_sb')
                    nc.sync.dma_start(
                        out=oar_sb,
                        in_=oar_dram[c].rearrange('(k p) s -> p k s', p=P)
                        [:, g * 8:(g + 1) * 8, :])
                    nc.vector.tensor_add(
                        h2T[:, g * 8:(g + 1) * 8, cs],
                        h2T[:, g * 8:(g + 1) * 8, cs], oar_sb)

            def stats6(c):
                cs = ts(c, TCH)
                acc2 = sq_chains(lambda k: h2T[:, k, cs], NDT, TCH, ph6,
                                 'ss2', 2)
                ss2 = sq_reduce(acc2, None, ph6, ph6ps, 'st_ps')
                r2 = finish_norm(ss2, 1.0 / D, f'r2_{c}')
                r2_b_c[c] = bcast_row(r2, f'r2_{c}', ph6, ph6ps)

            build_h2(0)
            stats6(0)
            # job order: LAG-tile c0 prologue, then interleave c1
            LAG = 2
            jobs = []
            for mcc in range(NMC):
                jobs.append((mcc, 0))
                if mcc >= LAG:
                    jobs.append((mcc - LAG, 1))
            for mcc in range(NMC - LAG, NMC):
                jobs.append((mcc, 1))
            woff = [mcc * NDT * P for mcc in range(NMC)]  # col offsets (els)
            wtiles = {}
            for mcc, ch in jobs:
                if ch == 1 and r2_b_c[1] is None:
                    build_h2(1)
                    stats6(1)
                cs = ts(ch, TCH)
                rows = min(P, IC - mcc * P)
                if mcc not in wtiles:
                    wg = ph6w.tile([P, NDT, P], bf16, tag='wg', bufs=3,
                                   name='wg')
                    wu = ph6w.tile([P, NDT, P], bf16, tag='wu', bufs=3,
                                   name='wu')
                    for wt, wsrc in ((wg, gate_own), (wu, up_own)):
                        for hh in range(2):
                            nc.sync.dma_start(
                                out=wt[:, hh * 16:(hh + 1) * 16, 0:rows],
                                in_=wsrc[:, ds(woff[mcc] + hh * 16 * rows,
                                               16 * rows)]
                                .rearrange('p (k n) -> p k n', n=rows))
                    wtiles[mcc] = (wg, wu)
                wg, wu = wtiles[mcc]
                ps_g = ph6ps.tile([P, TCH], f32, tag='g_ps', bufs=3, name='g_ps')
                ps_u = ph6ps.tile([P, TCH], f32, tag='u_ps', bufs=3, name='u_ps')
                for k in range(NDT):
                    mm(ps_g[0:rows], wg[:, k, 0:rows], h2T[:, k, cs],
                       k == 0, k == NDT - 1)
                    mm(ps_u[0:rows], wu[:, k, 0:rows], h2T[:, k, cs],
                       k == 0, k == NDT - 1)
                g = ph6.tile([P, TCH], f32, tag='g_sb', bufs=2, name='g_sb')
                nc.vector.tensor_mul(g[0:rows], ps_g[0:rows],
                                     r2_b_c[ch][0:rows])
                nc.scalar.activation(g[0:rows], g[0:rows], AF.Silu)
                u = ph6.tile([P, TCH], f32, tag='u_sb', bufs=2, name='u_sb')
                nc.vector.tensor_mul(u[0:rows], ps_u[0:rows],
                                     r2_b_c[ch][0:rows])
                nc.vector.tensor_mul(m_sb[0:rows, mcc, cs], g[0:rows],
                                     u[0:rows])

        # ============ phase 7: down partial + o fold + ReduceScatter =======
        # out-tile g order: halves {g%4<2} then {g%4>=2} so RS_A overlaps the
        # second half's GEMMs.  pd row layout: shard r' = g//4, block g%2.
        with tc.tile_pool(name='ph7', bufs=1) as ph7, \
             tc.tile_pool(name='ph7o', bufs=1) as ph7o, \
             tc.tile_pool(name='ph7ps', bufs=1, space='PSUM') as ph7ps:
            g_half = ([g for g in range(NDT) if g % 4 < 2],
                      [g for g in range(NDT) if g % 4 >= 2])
            for half in range(2):
                for g in g_half[half]:
                    w = ph7.tile([P, NMC, P], bf16, tag='dw', bufs=4, name='dw')
                    nc.sync.dma_start(
                        out=w, in_=down_own[:, ds(g * NMC * P, NMC * P)]
                        .rearrange('p (k n) -> p k n', n=P))
                    row0 = (g // 4) * 2 * P + (g % 2) * P
                    for ch in range(NCH):
                        cc_s = ts(ch, TCH)
                        ps = ph7ps.tile([P, TCH], f32, tag='d_ps', bufs=4,
                                        name='d_ps')
                        for k in range(NMC):
                            mm(ps, w[:, k, :], m_sb[:, k, cc_s],
                               k == 0, k == NMC - 1)
                        ore = ph7o.tile([P, TCH], bf16, tag='ore', bufs=4,
                                        name='ore')
                        nc.sync.dma_start(out=ore,
                                          in_=opart_dram[ch][ts(g, P), :])
                        pdt = ph7.tile([P, TCH], bf16, tag='pdt', bufs=4,
                                       name='pdt')
                        nc.vector.tensor_add(pdt, ps, ore)
                        nc.sync.dma_start(
                            out=pd_dram[half][ds(row0, P), cc_s], in_=pdt)
                cc('ReduceScatter', pd_dram[half], rs_dram[half])

            # final: out rows = h_own (fp32) + (o + mlp)_own from RS
            for half in range(2):
                rs_sb = ph7.tile([P, 2, S], bf16, tag='rs_sb', bufs=2,
                                 name='rs_sb')
                nc.sync.dma_start(
                    out=rs_sb,
                    in_=rs_dram[half].rearrange('(k p) s -> p k s', p=P))
                hre = ph7.tile([P, 2, S], f32, tag='hre', bufs=2, name='hre')
                nc.sync.dma_start(
                    out=hre, in_=h_ownD[ds(half * 2 * P, 2 * P), :]
                    .rearrange('(k p) s -> p k s', p=P))
                ot = ph7.tile([P, 2, S], f32, tag='of', bufs=2, name='of')
                nc.vector.tensor_add(ot, rs_sb, hre)
                for kk in range(2):
                    nc.sync.dma_start(
                        out=out[ds(half * 2 * P + kk * P, P), :],
                        in_=ot[:, kk, :])

    nc.compile()
    return nc


def _q8(w, scale):
    return np.clip(np.asarray(w, np.float32) * scale, -240, 240).astype(F8)


def _tileize(w, cols_slice=None):
    """[D_in, n] -> [P, D_in//P, n] contiguous fp32."""
    if cols_slice is not None:
        w = w[:, cols_slice]
    kin = w.shape[0] // P
    return np.ascontiguousarray(
        np.asarray(w, np.float32).reshape(kin, P, w.shape[1])
        .transpose(1, 0, 2))


def _flat_bf(w, cols_slice=None):
    """[D_in, n] -> [P, (D_in//P)*n] flat k-major blocks, bf16."""
    t = _tileize(w, cols_slice).astype(BF16)
    return np.ascontiguousarray(t.reshape(P, -1))


def _prep_inputs(inputs):
    h = np.ascontiguousarray(np.asarray(inputs['hidden_states'], np.float32))
    hT = np.ascontiguousarray(h.T)
    cosT = np.ascontiguousarray(np.asarray(inputs['cos'], np.float32).T)
    sinT = np.ascontiguousarray(np.asarray(inputs['sin'], np.float32).T)
    q_a_w = np.asarray(inputs['q_a_w'], np.float32)
    q_b_w = np.asarray(inputs['q_b_w'], np.float32)
    kv_a_w = np.asarray(inputs['kv_a_w'], np.float32)
    kv_b_w = np.asarray(inputs['kv_b_w'], np.float32)
    o_w = np.asarray(inputs['o_w'], np.float32)
    gate_w = np.asarray(inputs['gate_w'], np.float32)
    up_w = np.asarray(inputs['up_w'], np.float32)
    down_w = np.asarray(inputs['down_w'], np.float32)

    pidx = np.arange(P)[:, None]
    cidx = np.arange(TCH)[None, :]
    masks = np.stack([(cidx - pidx >= P * k) for k in range(4)]
                     ).astype(np.float32).astype(F8)

    cosT2 = np.ascontiguousarray(np.vstack([cosT, cosT]))
    sinT2 = np.ascontiguousarray(np.vstack([sinT, sinT]))
    R = np.zeros((DR, DR), np.float32)
    R[np.arange(DR // 2), np.arange(DR // 2) + DR // 2] = -1.0
    R[np.arange(DR // 2) + DR // 2, np.arange(DR // 2)] = 1.0
    R2 = np.zeros((P, P), np.float32)
    R2[:DR, :DR] = R
    R2[DR:, DR:] = R
    rot2T = np.ascontiguousarray(R2.T)

    # hT tiles [P, NDT, S] -> chunk-major [P, NCH, NDT, TCH]
    hT_t = _tileize(hT)
    hT_cm = np.ascontiguousarray(
        hT_t.reshape(P, NDT, NCH, TCH).transpose(0, 2, 1, 3))
    hT8 = np.clip(hT_cm * SH, -240, 240).astype(F8)
    hTb = hT_cm.astype(BF16)

    def gup_flat(w, r):
        blocks = []
        for mcc in range(NMC):
            rows = min(P, IC - mcc * P)
            blocks.append(_flat_bf(w, np.s_[r * IC + mcc * P:
                                            r * IC + mcc * P + rows]))
        return np.ascontiguousarray(np.concatenate(blocks, axis=1))

    in_maps = []
    for r in range(NC):
        heads = range(r * HPC, (r + 1) * HPC)
        qb_cols = np.concatenate(
            [q_b_w[:, hh * (DN + DR):hh * (DN + DR) + DN] for hh in heads] +
            [q_b_w[:, hh * (DN + DR) + DN:(hh + 1) * (DN + DR)] for hh in heads],
            axis=1)
        kvb_cols = np.concatenate(
            [kv_b_w[:, hh * (DN + DV):hh * (DN + DV) + DN] for hh in heads] +
            [kv_b_w[:, hh * (DN + DV) + DN:(hh + 1) * (DN + DV)] for hh in heads],
            axis=1)
        qb_blocks = np.stack(
            [np.ascontiguousarray(
                _tileize(qb_cols, np.s_[mc * P:(mc + 1) * P])
                .reshape(P, -1)) for mc in range(NQB)])
        # o_w rows for own heads: [HPC*DV, D] -> [P, HPC, D]
        o_rows = np.concatenate(
            [o_w[hh * DV:(hh + 1) * DV, :] for hh in heads], axis=0)
        # down_w rows for own IC (padded to NMC*P): [NMC*P, D] ->
        # [P, NMC, D] tiles -> g-major [P, NDT*NMC*P]
        d_rows = np.zeros((NMC * P, D), np.float32)
        d_rows[:IC] = down_w[r * IC:(r + 1) * IC, :]
        d_t = _tileize(d_rows)                       # [P, NMC, D]
        d_gm = np.ascontiguousarray(
            d_t.reshape(P, NMC, NDT, P).transpose(0, 2, 1, 3)
            .reshape(P, -1)).astype(BF16)
        in_maps.append({
            'hT8': hT8,
            'hTb': hTb,
            'h_ownD': np.ascontiguousarray(hT[r * OC:(r + 1) * OC]),
            'qa_own': _q8(_tileize(q_a_w, np.s_[r * QAC:(r + 1) * QAC]), SW),
            'kva_own': _q8(np.concatenate([_tileize(kv_a_w, np.s_[r * KVAC:(r + 1) * KVAC]), np.zeros((P, NDT, KVAP - KVAC), np.float32)], axis=2), SW),
            'qb_own': _q8(qb_blocks.reshape(NQB * P, NQLT * P), SW),
            'kvb_own': _q8(_tileize(kvb_cols), SW),
            'o_own': _q8(_tileize(o_rows), SW),
            'gate_own': gup_flat(gate_w, r),
            'up_own': gup_flat(up_w, r),
            'down_own': d_gm,
            'cosT2': cosT2.astype(BF16),
            'sinT2': sinT2.astype(BF16),
            'rot2T': rot2T.astype(BF16),
            'masks': masks,
        })
    return in_maps


def kernel(**inputs) -> np.ndarray:
    if 'nc' not in _CACHE:
        _CACHE['nc'] = _build()
    nc = _CACHE['nc']
    from concourse.bass_utils import run_bass_kernel_spmd
    in_maps = _prep_inputs(inputs)
    res = run_bass_kernel_spmd(nc, in_maps, core_ids=list(range(NC)))
    outT = np.concatenate([res.results[r]['out'] for r in range(NC)], axis=0)
    return np.ascontiguousarray(outT.T)


# revision 19
# speedup vs baseline: 1.0388x; 1.0388x over previous
"""DeepseekV3 decoder layer (MLA attention + dense MLP) on 8 trn2 NeuronCores.

v3: fp8 (e4m3) DoubleRow tensor-parallel kernel.

Changes vs v2 (AllGather-everywhere bf16):
- All attention-side GEMMs (q_a/kv_a, q_b/kv_b, v, probs@v, o_proj) run in
  fp8e4 with DoubleRow perf mode: 2 k-tiles per PE pass = 2x matmul
  throughput.  MLP GEMMs stay bf16 (fp8 error there lands 1:1 on the output
  and blows the 2e-2 budget; measured DoubleRow is 2x, so compensated
  schemes lose).
- o_proj flipped from AllGather(attn)+own-cols GEMM to own-heads partial
  GEMM + AllReduce: removes the attn AllGather round trip; the full h2 is
  rebuilt on every core (needed by the MLP anyway) as hT(bf16) + o_ar.
- down_proj flipped from AllGather(m)+own-cols GEMM to own-rows partial
  GEMM + ReduceScatter: m (22.5MB gathered before) never leaves SBUF.  The
  o partial is folded into the RS input, so RS directly yields
  (o + mlp)_own and the fp32 residual is added from the per-core h_ownD
  input (SPMD-safe: rank-dependence only through collectives / input data).
- probs quantize to fp8 straight out of Exp (max exp(SCALE*s) ~ 74 < 240);
  softmax denominator uses the same quantized probs so no rescaling needed.
- lq/lkv AllGathers carry fp8 (half the bytes).
"""
import sys

sys.path.insert(0, '/opt/trn_rl_repo')

import numpy as np
import ml_dtypes

S, D, H, QLORA, KVLORA = 1024, 4096, 32, 1536, 512
DN, DR, DV, INTER = 128, 64, 128, 11008
EPS = 1e-6
SCALE = (DN + DR) ** -0.5
NC = 8
HPC = H // NC               # 4 heads per core
QAC = QLORA // NC           # 192 q_a cols per core
KVAC = (KVLORA + DR) // NC  # 72 kv_a cols per core
KVAP = 80                   # padded to 16B stride for DoubleRow ldweights
OC = D // NC                # 512 output rows per core
IC = INTER // NC            # 1376 gate/up cols per core

P = 128
TCH = 512                   # token chunk
NCH = S // TCH              # 2 chunks
NDT = D // P                # 32
NKVT = KVLORA // P          # 4
NQLT = QLORA // P           # 12
NTT = S // P                # 8
NQB = HPC * (DN + DR) // P  # 6 qT row chunks (4 nope + 2 pe)
NOB = OC // P               # 4
NMC = (IC + P - 1) // P     # 11 gate/up row tiles (last is 96)
BF16 = ml_dtypes.bfloat16
F8 = ml_dtypes.float8_e4m3

# fp8 scales
SH = 16.0     # hT8 = fp8(SH * h)
SW = 256.0    # fp8 weights = fp8(SW * w)
SLQ = 16.0    # lq/lkv written as fp8(SLQ * lq_raw)
SV = 16.0     # v_sb = fp8(SV * v_true); attn then = SV * attn_true
S2 = SW * SLQ  # 4096: psum scale of fp8 b-projections

_CACHE = {}


def _build():
    import concourse.bass as bass
    import concourse.tile as tile
    from concourse import bacc, mybir
    from contextlib import ExitStack

    dt = mybir.dt
    f32, bf16, f8 = dt.float32, dt.bfloat16, dt.float8e4
    AF = mybir.ActivationFunctionType
    DR_MODE = mybir.MatmulPerfMode.DoubleRow
    ts, ds = bass.ts, bass.ds

    nc = bacc.Bacc('TRN2', target_bir_lowering=False, debug=False,
                   num_devices=NC)

    hT8 = nc.dram_tensor('hT8', [P, NCH, NDT, TCH], f8, kind='ExternalInput')
    hTb = nc.dram_tensor('hTb', [P, NCH, NDT, TCH], bf16, kind='ExternalInput')
    h_ownD = nc.dram_tensor('h_ownD', [OC, S], f32, kind='ExternalInput')
    qa_own = nc.dram_tensor('qa_own', [P, NDT, QAC], f8, kind='ExternalInput')
    kva_own = nc.dram_tensor('kva_own', [P, NDT, KVAP], f8, kind='ExternalInput')
    qb_own = nc.dram_tensor('qb_own', [NQB * P, NQLT * P], f8, kind='ExternalInput')
    kvb_own = nc.dram_tensor('kvb_own', [P, NKVT, HPC * (DN + DV)], f8,
                             kind='ExternalInput')
    o_own = nc.dram_tensor('o_own', [P, HPC, D], f8, kind='ExternalInput')
    gate_own = nc.dram_tensor('gate_own', [P, NDT * IC], bf16, kind='ExternalInput')
    up_own = nc.dram_tensor('up_own', [P, NDT * IC], bf16, kind='ExternalInput')
    down_own = nc.dram_tensor('down_own', [P, NDT * NMC * P], bf16,
                              kind='ExternalInput')
    cosT_d = nc.dram_tensor('cosT2', [P, S], bf16, kind='ExternalInput')
    sinT_d = nc.dram_tensor('sinT2', [P, S], bf16, kind='ExternalInput')
    rot2_d = nc.dram_tensor('rot2T', [P, P], bf16, kind='ExternalInput')
    masks_d = nc.dram_tensor('masks', [4, P, TCH], f8, kind='ExternalInput')
    out = nc.dram_tensor('out', [OC, S], f32, kind='ExternalOutput')

    RG = [list(range(NC))]

    def mm(psum, lhsT, rhs, start, stop):
        nc.tensor.matmul(psum, lhsT, rhs, start=start, stop=stop)

    def mm2(psum, lhsT, rhs, start, stop):
        nc.tensor.matmul(psum, lhsT, rhs, start=start, stop=stop,
                         perf_mode=DR_MODE)

    def cc(kind, in_t, out_t, op=None):
        op = op or (mybir.AluOpType.bypass if kind == 'AllGather'
                    else mybir.AluOpType.add)
        nc.gpsimd.collective_compute(
            kind, op, replica_groups=RG, ins=[in_t[:]], outs=[out_t[:]])

    with tile.TileContext(nc) as tc, ExitStack() as st:
        const = st.enter_context(tc.tile_pool(name='const', bufs=1))
        vecs = st.enter_context(tc.tile_pool(name='vecs', bufs=1))
        dram = st.enter_context(tc.tile_pool(name='dram', bufs=1, space='DRAM'))

        ones_bf = const.tile([P, 1], bf16)
        nc.vector.memset(ones_bf, 1.0)
        ones8w = const.tile([P, 2, 16], f8)
        nc.vector.memset(ones8w, 1.0)
        ones8 = ones8w[:, :, 0:1]
        onesrow_bf = const.tile([1, P], bf16)
        nc.vector.memset(onesrow_bf, 1.0)
        eps1 = const.tile([1, 1], f32)
        nc.vector.memset(eps1, EPS)
        epsq = const.tile([1, 1], f32)
        nc.vector.memset(epsq, EPS * S2 * S2)

        lq_dram = [dram.tile([QAC, TCH], f8, name=f'lq_dram{c}')
                   for c in range(NCH)]
        lq_ag = [dram.tile([QLORA, TCH], f8, addr_space='Shared',
                           name=f'lq_ag{c}') for c in range(NCH)]
        lkv_dram = [dram.tile([KVAC, TCH], f8, name=f'lkv_dram{c}')
                    for c in range(NCH)]
        lkv_ag = [dram.tile([KVLORA + DR, TCH], f8, addr_space='Shared',
                            name=f'lkv_ag{c}') for c in range(NCH)]
        opart_dram = [dram.tile([D, TCH], bf16, name=f'opart{c}')
                      for c in range(NCH)]
        oar_dram = [dram.tile([D, TCH], bf16, addr_space='Shared',
                              name=f'oar{c}') for c in range(NCH)]
        pd_dram = [dram.tile([NC * 3 * P, S], bf16, name='pdA'),
                   dram.tile([NC * P, S], bf16, name='pdB')]
        rs_dram = [dram.tile([3 * P, S], bf16, name='rsA'),
                   dram.tile([P, S], bf16, name='rsB')]

        # ---- helpers ----------------------------------------------------
        def vrow(name):
            return vecs.tile([1, TCH], f32, tag='vrow', bufs=4, name=name)

        def bcast_row(row_fp32, name, pool, ps_pool, ps_bufs=1, bufs=1):
            """[1,TCH] fp32 -> [P,TCH] fp32 SBUF (bf16 precision) via matmul."""
            rb = pool.tile([1, TCH], bf16, tag='brow', bufs=3, name=f'{name}_r')
            nc.vector.tensor_copy(rb, row_fp32)
            ps = ps_pool.tile([P, TCH], f32, tag='bc_ps', bufs=ps_bufs,
                              name=f'{name}_ps')
            mm(ps, onesrow_bf, rb[0:1, :], True, True)
            sb = pool.tile([P, TCH], f32, tag=f'{name}_bc', bufs=bufs,
                           name=f'{name}_bc')
            nc.vector.tensor_copy(sb, ps)
            return sb

        def finish_norm(ps_sum, scale_meanN, name, extra_sq=None, tag='vrow',
                        bias=None):
            sb = vecs.tile([1, TCH], f32, tag=tag, bufs=4, name=name)
            if extra_sq is not None:
                nc.vector.tensor_mul(sb, ps_sum, extra_sq)
            else:
                nc.vector.tensor_copy(sb, ps_sum)
            nc.scalar.activation(sb, sb, AF.Sqrt, bias=bias if bias is not None
                                 else eps1, scale=scale_meanN)
            nc.vector.reciprocal_approx_fast(out=sb, in_=sb)
            return sb

        def sq_chains(get_src, n, width, pool, tag, nacc, k_lo=0, k_hi=None,
                      accs=None):
            """acc[a] accumulates get_src(k)^2 (ACT square + DVE adds)."""
            if k_hi is None:
                k_hi = n
            if accs is None:
                accs = [pool.tile([P, width], f32, tag=f'{tag}a{a}', bufs=1,
                                  name=f'{tag}a{a}') for a in range(nacc)]
            for k in range(k_lo, k_hi):
                a = k % nacc
                if k < nacc:
                    nc.scalar.activation(accs[a], get_src(k), AF.Square)
                else:
                    sq = pool.tile([P, width], f32, tag=f'{tag}s', bufs=2,
                                   name=f'{tag}s')
                    nc.scalar.activation(sq, get_src(k), AF.Square)
                    nc.vector.tensor_add(accs[a], accs[a], sq)
            return accs

        def sq_reduce(accs, cs, pool, ps_pool, tag, ps_bufs=1):
            ps = ps_pool.tile([1, TCH], f32, tag=tag, bufs=ps_bufs, name=tag)
            for a, acc in enumerate(accs):
                ab = pool.tile([P, TCH], bf16, tag='accb', bufs=2, name='accb')
                nc.vector.tensor_copy(ab, acc[:, cs] if cs is not None else acc)
                mm(ps, ones_bf, ab, a == 0, a == len(accs) - 1)
            return ps

        # warmup collective: pays the first-CC barrier/ramp cost early
        warm_in = dram.tile([P, 16], f8, name='warm_in')
        warm_out = dram.tile([NC * P, 16], f8, addr_space='Shared',
                             name='warm_out')
        warm_sb = const.tile([P, 16], f8)
        nc.vector.memset(warm_sb, 0.0)
        nc.sync.dma_start(out=warm_in[:], in_=warm_sb)
        cc('AllGather', warm_in, warm_out)

        # ---- persistent SBUF --------------------------------------------
        r1_c = [None, None]
        cosr1_c, sinr1_c = [None, None], [None, None]
        r2_b_c = [None, None]

        mlp_sb = st.enter_context(tc.tile_pool(name='mlp_sb', bufs=1))
        h2T = mlp_sb.tile([P, NDT, S], bf16, name='h2T')

        with ExitStack() as att_scope:
            attp = att_scope.enter_context(tc.tile_pool(name='attp', bufs=1))
            qT = attp.tile([P, NQB, S], bf16, name='qT')
            kT = attp.tile([P, HPC, S], bf16, name='kT')
            v_sb = attp.tile([P, NTT, HPC * DV], f8, name='v_sb')
            kpe = attp.tile([P, S], bf16, name='kpe')
            cos_sb = attp.tile([P, S], bf16, name='cos_sb')
            nc.sync.dma_start(out=cos_sb, in_=cosT_d[:])
            sin_sb = attp.tile([P, S], bf16, name='sin_sb')
            nc.sync.dma_start(out=sin_sb, in_=sinT_d[:])
            rot2_sb = attp.tile([P, P], bf16, name='rot2_sb')
            nc.sync.dma_start(out=rot2_sb, in_=rot2_d[:])
            masks_sb = attp.tile([P, 4, TCH], f8, name='masks_sb')
            nc.sync.dma_start(out=masks_sb, in_=masks_d.rearrange('m p c -> p m c'))
            qa_sb = attp.tile([P, NDT, QAC], f8, name='qa_sb')
            nc.sync.dma_start(out=qa_sb, in_=qa_own[:])
            kva_sb = attp.tile([P, NDT, KVAP], f8, name='kva_sb')
            nc.sync.dma_start(out=kva_sb, in_=kva_own[:])
            kvb_sb = attp.tile([P, NKVT, HPC * (DN + DV)], f8, name='kvb_sb')
            nc.sync.dma_start(out=kvb_sb, in_=kvb_own[:])

            # ============ phase 1: a-projections + input-norm stats =======
            with ExitStack() as ph1_scope:
                hkp = ph1_scope.enter_context(tc.tile_pool(name='hkp', bufs=1))
                ph1 = ph1_scope.enter_context(tc.tile_pool(name='ph1', bufs=1))
                ph1ps = ph1_scope.enter_context(
                    tc.tile_pool(name='ph1ps', bufs=1, space='PSUM'))
                G1 = 8
                ss_acc_c = [None, None]
                for c in range(NCH):
                    hk = hkp.tile([P, NDT, TCH], f8, tag='hk', bufs=2,
                                  name='hk')
                    for g in range(NDT // G1):
                        nc.sync.dma_start(
                            out=hk[:, g * G1:(g + 1) * G1, :],
                            in_=hT8[:, c, g * G1:(g + 1) * G1, :])
                    ps1 = ph1ps.tile([P, TCH], f32, tag='lq1', bufs=2, name='lq1')
                    ps2 = ph1ps.tile([QAC - P, TCH], f32, tag='lq2', bufs=1,
                                     name='lq2')
                    for k in range(0, NDT, 2):
                        mm2(ps1, qa_sb[:, k:k + 2, 0:P], hk[:, k:k + 2, :],
                            k == 0, k == NDT - 2)
                        mm2(ps2, qa_sb[:, k:k + 2, P:QAC], hk[:, k:k + 2, :],
                            k == 0, k == NDT - 2)
                    lq1 = ph1.tile([P, TCH], f8, tag='lq1s', bufs=2, name='lq1s')
                    nc.vector.tensor_scalar_mul(lq1, ps1, SLQ / (SH * SW))
                    nc.sync.dma_start(out=lq_dram[c][0:P, :], in_=lq1)
                    lq2 = ph1.tile([QAC - P, TCH], f8, tag='lq2s', bufs=2,
                                   name='lq2s')
                    nc.vector.tensor_scalar_mul(lq2, ps2, SLQ / (SH * SW))
                    nc.sync.dma_start(out=lq_dram[c][P:QAC, :], in_=lq2)
                    cc('AllGather', lq_dram[c], lq_ag[c])
                    pskv = ph1ps.tile([KVAP, TCH], f32, tag='lkv', bufs=1,
                                      name='lkv')
                    for k in range(0, NDT, 2):
                        mm2(pskv, kva_sb[:, k:k + 2, :], hk[:, k:k + 2, :],
                            k == 0, k == NDT - 2)
                    lkv1 = ph1.tile([KVAC, TCH], f8, tag='lkvs', bufs=2,
                                    name='lkvs')
                    nc.vector.tensor_scalar_mul(lkv1, pskv[0:KVAC, :], SLQ / (SH * SW))
                    nc.sync.dma_start(out=lkv_dram[c][:], in_=lkv1)
                    cc('AllGather', lkv_dram[c], lkv_ag[c])
                    ss_acc_c[c] = sq_chains(
                        lambda k: hk[:, k, :], NDT, TCH, ph1, f'ss{c}', 2)
                # input-norm factors per chunk (ss holds SH^2 * h^2 sums)
                for c in range(NCH):
                    cs = ts(c, TCH)
                    ss = sq_reduce(ss_acc_c[c], None, ph1,
                                   ph1ps, 'ss_ps', ps_bufs=2)
                    r1 = finish_norm(ss, 1.0 / (D * SH * SH), f'r1_{c}',
                                     tag='r1')
                    r1sq = vecs.tile([1, TCH], f32, tag='r1sq', bufs=2,
                                     name='r1sq')
                    nc.vector.tensor_mul(r1sq, r1, r1)
                    r1_c[c] = (r1, r1sq)
                    # kpe factors need r1/SLQ (lkv_ag is fp8 = SLQ*lkv_raw)
                    r1d = vecs.tile([1, TCH], f32, tag='r1d', bufs=2,
                                    name='r1d')
                    nc.vector.tensor_scalar_mul(r1d, r1, 1.0 / SLQ)
                    r1b = bcast_row(r1d, f'r1_{c}', ph1, ph1ps, ps_bufs=2,
                                    bufs=1)
                    cr = attp.tile([P, TCH], bf16, tag='cosr1', bufs=2,
                                   name='cosr1')
                    nc.vector.tensor_mul(cr, cos_sb[:, cs], r1b)
                    sr = attp.tile([P, TCH], bf16, tag='sinr1', bufs=2,
                                   name='sinr1')
                    nc.vector.tensor_mul(sr, sin_sb[:, cs], r1b)
                    cosr1_c[c], sinr1_c[c] = cr, sr

            # ============ phases 2-5 per chunk ============================
            pre = att_scope.enter_context(tc.tile_pool(name='pre', bufs=1))
            o_sb = pre.tile([P, HPC, D], f8, name='o_sb')
            nc.sync.dma_start(out=o_sb, in_=o_own[:])
            for c in range(NCH):
                cs = ts(c, TCH)
                r1, r1sq = r1_c[c]
                with tc.tile_pool(name='ph2', bufs=1) as ph2, \
                     tc.tile_pool(name='ph2w', bufs=3) as ph2w, \
                     tc.tile_pool(name='ph2ps', bufs=1, space='PSUM') as ph2ps:
                    # prefetch this chunk's residual rows into h2T early;
                    # the o_ar add happens lazily before the MLP needs it
                    nc.sync.dma_start(out=h2T[:, :, cs], in_=hTb[:, c, :, :])
                    lqn = pre.tile([P, NQLT, TCH], f8, tag='lqn', bufs=1,
                                   name='lqn')
                    for g in range(2):
                        nc.sync.dma_start(
                            out=lqn[:, g * 6:(g + 1) * 6, :],
                            in_=lq_ag[c].rearrange('(k p) s -> p k s', p=P)
                            [:, g * 6:(g + 1) * 6, :])
                    kvn = pre.tile([P, NKVT, TCH], f8, tag='kvn', bufs=1,
                                   name='kvn')
                    nc.sync.dma_start(
                        out=kvn, in_=lkv_ag[c][0:KVLORA, :]
                        .rearrange('(k p) s -> p k s', p=P))

                    # q_b GEMM mc 0-2 (PSUM qb_ps: 3 banks)
                    def qbw_tile(mc2):
                        w = ph2w.tile([P, NQLT, P], f8, tag='qbw', bufs=6,
                                      name='qbw')
                        nc.sync.dma_start(
                            out=w, in_=qb_own[ds(mc2 * P, P), :]
                            .rearrange('p (k n) -> p k n', n=P))
                        return w

                    ps_q = []
                    for mc2 in range(3):
                        w = qbw_tile(mc2)
                        ps = ph2ps.tile([P, TCH], f32, tag='qb_ps', bufs=3,
                                        name='qb_ps')
                        for k in range(0, NQLT, 2):
                            mm2(ps, w[:, k:k + 2, :], lqn[:, k:k + 2, :],
                                k == 0, k == NQLT - 2)
                        ps_q.append(ps)
                    # rq' = 1/(S2*sqrt(mean(lq^2)+eps)); fq = rq'*r1
                    acc_q = sq_chains(lambda k: lqn[:, k, :], NQLT, TCH, ph2,
                                      'st2', 2)
                    ssq = sq_reduce(acc_q, None, ph2, ph2ps, 'st_ps')
                    rq = finish_norm(ssq, (S2 * S2) / (QLORA * SLQ * SLQ),
                                     f'rq_{c}', extra_sq=r1sq, bias=epsq)
                    fq = vrow(f'fq_{c}')
                    nc.vector.tensor_mul(fq, rq, r1)
                    fq_b = bcast_row(fq, f'fq_{c}', ph2, ph2ps)
                    cf = ph2.tile([P, TCH], bf16, tag='cosfq', bufs=1,
                                  name='cosfq')
                    nc.vector.tensor_mul(cf, cos_sb[:, cs], fq_b)
                    sf = ph2.tile([P, TCH], bf16, tag='sinfq', bufs=1,
                                  name='sinfq')
                    nc.vector.tensor_mul(sf, sin_sb[:, cs], fq_b)
                    for mc2 in range(3):
                        nc.vector.tensor_mul(qT[:, mc2, cs], ps_q[mc2], fq_b)
                    # remaining q_b tiles: mc 3 (nope) + 4,5 (pe with rope)
                    for mc2 in range(3, NQB):
                        w = qbw_tile(mc2)
                        ps = ph2ps.tile([P, TCH], f32, tag='qb_ps', bufs=3,
                                        name='qb_ps')
                        for k in range(0, NQLT, 2):
                            mm2(ps, w[:, k:k + 2, :], lqn[:, k:k + 2, :],
                                k == 0, k == NQLT - 2)
                        if mc2 == 3:
                            nc.vector.tensor_mul(qT[:, mc2, cs], ps, fq_b)
                        else:
                            qraw = ph2.tile([P, TCH], bf16, tag='qraw', bufs=1,
                                            name='qraw')
                            nc.vector.tensor_copy(qraw, ps)
                            ps2 = ph2ps.tile([P, TCH], f32, tag='qrot', bufs=1,
                                             name='qrot')
                            nc.tensor.matmul(ps2, rot2_sb, qraw,
                                             start=True, stop=True)
                            rot_s = ph2.tile([P, TCH], f32, tag='rot_qs',
                                             bufs=1, name='rot_qs')
                            nc.vector.tensor_mul(rot_s, ps2, sf)
                            nc.vector.tensor_mul(qT[:, mc2, cs], qraw, cf)
                            nc.vector.tensor_add(qT[:, mc2, cs],
                                                 qT[:, mc2, cs], rot_s)

                    # kv stats: rkv true (for requant) + /S2 variant (kT)
                    acc_kv = sq_chains(lambda k: kvn[:, k, :], NKVT, TCH, ph2,
                                       'st2', 2)
                    sskv = sq_reduce(acc_kv, None, ph2, ph2ps, 'st_ps')
                    rkv = finish_norm(sskv, 1.0 / (KVLORA * SLQ * SLQ),
                                      f'rkv_{c}', extra_sq=r1sq)
                    fkvv = vrow(f'fkvv_{c}')
                    nc.vector.tensor_mul(fkvv, rkv, r1)
                    fkv = vrow(f'fkv_{c}')
                    nc.vector.tensor_scalar_mul(fkv, fkvv, 1.0 / S2)
                    fkv_b = bcast_row(fkv, f'fkv_{c}', ph2, ph2ps)
                    # kT on raw kvn, drain-scaled (PSUM kv_ps: 2 banks)
                    for j in range(HPC):
                        ps = ph2ps.tile([P, TCH], f32, tag='kv_ps', bufs=2,
                                        name='kv_ps')
                        for k in range(0, NKVT, 2):
                            mm2(ps, kvb_sb[:, k:k + 2, ts(j, DN)],
                                kvn[:, k:k + 2, :], k == 0, k == NKVT - 2)
                        nc.vector.tensor_mul(kT[:, j, cs], ps, fkv_b)
                    # requantize kvn with the norm factors for the v GEMM
                    fkvv_b = bcast_row(fkvv, f'fkvv_{c}', ph2, ph2ps)
                    kvs = ph2.tile([P, NKVT, TCH], f8, tag='kvs', bufs=1,
                                   name='kvs')
                    for k in range(NKVT):
                        nc.vector.tensor_mul(kvs[:, k, :], kvn[:, k, :], fkvv_b)
                    for i in range(4 * c, 4 * c + 4):
                        il = i - 4 * c
                        ps = ph2ps.tile([P, HPC * DV], f32, tag='kv_ps', bufs=2,
                                        name='kv_ps')
                        for k in range(0, NKVT, 2):
                            mm2(ps, kvs[:, k:k + 2, ts(il, P)],
                                kvb_sb[:, k:k + 2, HPC * DN:],
                                k == 0, k == NKVT - 2)
                        nc.vector.tensor_scalar_mul(v_sb[:, i, :], ps, SV / S2)
                    # k_pe rope: kpe = raw*(cos*r1/SLQ) + rot(raw)*(sin*r1/SLQ)
                    kpe_raw8 = ph2.tile([DR, TCH], f8, tag='kpe_raw8', bufs=1,
                                        name='kpe_raw8')
                    nc.sync.dma_start(out=kpe_raw8,
                                      in_=lkv_ag[c][KVLORA:KVLORA + DR, :])
                    kpe_raw = ph2.tile([DR, TCH], bf16, tag='kpe_raw', bufs=1,
                                       name='kpe_raw')
                    nc.vector.tensor_copy(kpe_raw, kpe_raw8)
                    ps_rot = ph2ps.tile([P, TCH], f32, tag='qrot', bufs=1,
                                        name='kperot')
                    nc.tensor.matmul(ps_rot[0:DR, :], rot2_sb[0:DR, 0:DR],
                                     kpe_raw, start=True, stop=True)
                    rot_s = ph2.tile([DR, TCH], f32, tag='kpe_rs', bufs=1,
                                     name='kpe_rs')
                    nc.vector.tensor_mul(rot_s, ps_rot[0:DR, :],
                                         sinr1_c[c][0:DR, :])
                    kpe_t = ph2.tile([DR, TCH], f32, tag='kpe_t', bufs=1,
                                     name='kpe_t')
                    nc.vector.tensor_mul(kpe_t, kpe_raw, cosr1_c[c][0:DR, :])
                    nc.vector.tensor_add(kpe[0:DR, cs], kpe_t, rot_s)
                    nc.sync.dma_start(out=kpe[DR:P, cs], in_=kpe[0:DR, cs])

                # ---- attention for this chunk ----------------------------
                # PSUM: sc 2 + se 2 + at 2 + bc 2 = 8 banks
                attn_loc = None
                with tc.tile_pool(name='ph4', bufs=1) as ph4, \
                     tc.tile_pool(name='ph4p', bufs=1) as ph4p, \
                     tc.tile_pool(name='ph4ps', bufs=1, space='PSUM') as ph4ps:
                    attn_loc = pre.tile([P, HPC, TCH], f8, tag='attn_loc',
                                        bufs=2, name='attn_loc')
                    ilist = list(range(4 * c + 4))
                    npair = len(ilist) // 2
                    for j in range(HPC):
                        pe_mc = HPC * DN // P + (j * DR) // P
                        pe_off = (j * DR) % P
                        epairs = []
                        for n in range(npair):
                            ep = ph4p.tile([P, 2, TCH], f8, tag=f'probs{n}',
                                           bufs=2, name=f'probs{n}')
                            for half in range(2):
                                i = 2 * n + half
                                ps = ph4ps.tile([P, TCH], f32, tag='sc_ps',
                                                bufs=2, name='sc_ps')
                                mm(ps, kT[:, j, ts(i, P)], qT[:, j, cs],
                                   True, False)
                                mm(ps, kpe[pe_off:pe_off + DR, ts(i, P)],
                                   qT[pe_off:pe_off + DR, pe_mc, cs],
                                   False, True)
                                nc.scalar.activation(ep[:, half, :], ps, AF.Exp,
                                                     scale=SCALE)
                                if i // 4 == c:
                                    nc.vector.tensor_mul(
                                        ep[:, half, :], ep[:, half, :],
                                        masks_sb[:, i % 4, :])
                            epairs.append(ep)
                        ps_se = ph4ps.tile([1, TCH], f32, tag='se_ps', bufs=2,
                                           name='se_ps')
                        for n, ep in enumerate(epairs):
                            mm2(ps_se, ones8, ep, n == 0, n == npair - 1)
                        ps_at = ph4ps.tile([P, TCH], f32, tag='at_ps', bufs=2,
                                           name='at_ps')
                        for n, ep in enumerate(epairs):
                            mm2(ps_at, v_sb[:, 2 * n:2 * n + 2, ts(j, DV)], ep,
                                n == 0, n == npair - 1)
                        recip = vrow(f'recip_{c}_{j}')
                        sef = vrow(f'se_{c}_{j}')
                        nc.vector.tensor_copy(sef, ps_se)
                        nc.vector.reciprocal_approx_fast(out=recip, in_=sef)
                        recip_b = bcast_row(recip, 'recip', ph4, ph4ps,
                                            ps_bufs=2, bufs=1)
                        nc.vector.tensor_mul(attn_loc[:, j, :], ps_at, recip_b)

                # ---- o_proj partial (own heads) + AllReduce --------------
                with tc.tile_pool(name='ph5', bufs=1) as ph5, \
                     tc.tile_pool(name='ph5ps', bufs=1, space='PSUM') as ph5ps:
                    for g in range(NDT):
                        ps = ph5ps.tile([P, TCH], f32, tag='o_ps', bufs=4,
                                        name='o_ps')
                        mm2(ps, o_sb[:, 0:2, ts(g, P)], attn_loc[:, 0:2, :],
                            True, False)
                        mm2(ps, o_sb[:, 2:4, ts(g, P)], attn_loc[:, 2:4, :],
                            False, True)
                        ot = ph5.tile([P, TCH], bf16, tag='ot', bufs=4,
                                      name='ot')
                        nc.vector.tensor_scalar_mul(ot, ps, 1.0 / (SW * SV))
                        nc.sync.dma_start(out=opart_dram[c][ts(g, P), :],
                                          in_=ot)
                cc('AllReduce', opart_dram[c], oar_dram[c])

        # ============ phase 6: post-norm stats + gate/up (lag pipeline) ====
        msp = st.enter_context(tc.tile_pool(name='msp', bufs=1))
        m_sb = msp.tile([P, NMC, S], bf16, name='m_sb')
        nc.vector.memset(m_sb[96:P, NMC - 1, :], 0.0)
        with ExitStack() as mlp_scope:
            ph6 = mlp_scope.enter_context(tc.tile_pool(name='ph6', bufs=1))
            ph6w = mlp_scope.enter_context(tc.tile_pool(name='ph6w', bufs=1))
            ph6ps = mlp_scope.enter_context(
                tc.tile_pool(name='ph6ps', bufs=1, space='PSUM'))

            def build_h2(c):
                # h2T[:, :, cs] += o_ar (residual rows were DMA'd in ph2)
                cs = ts(c, TCH)
                for g in range(NDT // 8):
                    oar_sb = ph6.tile([P, 8, TCH], bf16, tag='oar_sb', bufs=2,
                                      name='oar_sb')
                    nc.sync.dma_start(
                        out=oar_sb,
                        in_=oar_dram[c].rearrange('(k p) s -> p k s', p=P)
                        [:, g * 8:(g + 1) * 8, :])
                    nc.vector.tensor_add(
                        h2T[:, g * 8:(g + 1) * 8, cs],
                        h2T[:, g * 8:(g + 1) * 8, cs], oar_sb)

            def stats6(c):
                cs = ts(c, TCH)
                acc2 = sq_chains(lambda k: h2T[:, k, cs], NDT, TCH, ph6,
                                 'ss2', 2)
                ss2 = sq_reduce(acc2, None, ph6, ph6ps, 'st_ps')
                r2 = finish_norm(ss2, 1.0 / D, f'r2_{c}')
                r2_b_c[c] = bcast_row(r2, f'r2_{c}', ph6, ph6ps)

            build_h2(0)
            stats6(0)
            # all c0 jobs first, then c1 (c1's o-AllReduce lands meanwhile);
            # weights are re-streamed per chunk
            jobs = [(m, 0) for m in range(NMC)] + [(m, 1) for m in range(NMC)]
            woff = [mcc * NDT * P for mcc in range(NMC)]  # col offsets (els)
            for mcc, ch in jobs:
                if ch == 1 and r2_b_c[1] is None:
                    build_h2(1)
                    stats6(1)
                cs = ts(ch, TCH)
                rows = min(P, IC - mcc * P)
                wg = ph6w.tile([P, NDT, P], bf16, tag='wg', bufs=3,
                               name='wg')
                wu = ph6w.tile([P, NDT, P], bf16, tag='wu', bufs=3,
                               name='wu')
                for wt, wsrc in ((wg, gate_own), (wu, up_own)):
                    for hh in range(2):
                        nc.sync.dma_start(
                            out=wt[:, hh * 16:(hh + 1) * 16, 0:rows],
                            in_=wsrc[:, ds(woff[mcc] + hh * 16 * rows,
                                           16 * rows)]
                            .rearrange('p (k n) -> p k n', n=rows))
                ps_g = ph6ps.tile([P, TCH], f32, tag='g_ps', bufs=3, name='g_ps')
                ps_u = ph6ps.tile([P, TCH], f32, tag='u_ps', bufs=3, name='u_ps')
                for k in range(NDT):
                    mm(ps_g[0:rows], wg[:, k, 0:rows], h2T[:, k, cs],
                       k == 0, k == NDT - 1)
                    mm(ps_u[0:rows], wu[:, k, 0:rows], h2T[:, k, cs],
                       k == 0, k == NDT - 1)
                g = ph6.tile([P, TCH], f32, tag='g_sb', bufs=2, name='g_sb')
                nc.vector.tensor_mul(g[0:rows], ps_g[0:rows],
                                     r2_b_c[ch][0:rows])
                nc.scalar.activation(g[0:rows], g[0:rows], AF.Silu)
                u = ph6.tile([P, TCH], f32, tag='u_sb', bufs=2, name='u_sb')
                nc.vector.tensor_mul(u[0:rows], ps_u[0:rows],
                                     r2_b_c[ch][0:rows])
                nc.vector.tensor_mul(m_sb[0:rows, mcc, cs], g[0:rows],
                                     u[0:rows])

        # ============ phase 7: down partial + o fold + ReduceScatter =======
        # out-tile g order: halves {g%4<2} then {g%4>=2} so RS_A overlaps the
        # second half's GEMMs.  pd row layout: shard r' = g//4, block g%2.
        with tc.tile_pool(name='ph7', bufs=1) as ph7, \
             tc.tile_pool(name='ph7o', bufs=1) as ph7o, \
             tc.tile_pool(name='ph7ps', bufs=1, space='PSUM') as ph7ps:
            g_half = ([g for g in range(NDT) if g % 4 < 3],
                      [g for g in range(NDT) if g % 4 == 3])
            nblk = (3, 1)

            def final_add(half):
                # out rows = h_own (fp32) + (o + mlp)_own from RS
                nb = nblk[half]
                r0 = 0 if half == 0 else 3 * P
                rs_sb = ph7.tile([P, nb, S], bf16, tag=f'rs_sb{half}', bufs=1,
                                 name='rs_sb')
                nc.sync.dma_start(
                    out=rs_sb,
                    in_=rs_dram[half].rearrange('(k p) s -> p k s', p=P))
                hre = ph7.tile([P, nb, S], f32, tag=f'hre{half}', bufs=1,
                               name='hre')
                nc.sync.dma_start(
                    out=hre, in_=h_ownD[ds(r0, nb * P), :]
                    .rearrange('(k p) s -> p k s', p=P))
                ot = ph7.tile([P, nb, S], f32, tag=f'of{half}', bufs=1,
                              name='of')
                nc.vector.tensor_add(ot, rs_sb, hre)
                for kk in range(nb):
                    nc.sync.dma_start(
                        out=out[ds(r0 + kk * P, P), :], in_=ot[:, kk, :])

            for half in range(2):
                for g in g_half[half]:
                    w = ph7.tile([P, NMC, P], bf16, tag='dw', bufs=4, name='dw')
                    nc.sync.dma_start(
                        out=w, in_=down_own[:, ds(g * NMC * P, NMC * P)]
                        .rearrange('p (k n) -> p k n', n=P))
                    row0 = ((g // 4) * nblk[half] + (g % 4 if half == 0
                                                    else 0)) * P
                    for ch in range(NCH):
                        cc_s = ts(ch, TCH)
                        ps = ph7ps.tile([P, TCH], f32, tag='d_ps', bufs=4,
                                        name='d_ps')
                        for k in range(NMC):
                            mm(ps, w[:, k, :], m_sb[:, k, cc_s],
                               k == 0, k == NMC - 1)
                        ore = ph7o.tile([P, TCH], bf16, tag='ore', bufs=4,
                                        name='ore')
                        nc.sync.dma_start(out=ore,
                                          in_=opart_dram[ch][ts(g, P), :])
                        pdt = ph7.tile([P, TCH], bf16, tag='pdt', bufs=4,
                                       name='pdt')
                        nc.vector.tensor_add(pdt, ps, ore)
                        nc.sync.dma_start(
                            out=pd_dram[half][ds(row0, P), cc_s], in_=pdt)
                cc('ReduceScatter', pd_dram[half], rs_dram[half])
                if half == 1:
                    final_add(0)
            final_add(1)

    nc.compile()
    return nc


def _q8(w, scale):
    return np.clip(np.asarray(w, np.float32) * scale, -240, 240).astype(F8)


def _tileize(w, cols_slice=None):
    """[D_in, n] -> [P, D_in//P, n] contiguous fp32."""
    if cols_slice is not None:
        w = w[:, cols_slice]
    kin = w.shape[0] // P
    return np.ascontiguousarray(
        np.asarray(w, np.float32).reshape(kin, P, w.shape[1])
        .transpose(1, 0, 2))


def _flat_bf(w, cols_slice=None):
    """[D_in, n] -> [P, (D_in//P)*n] flat k-major blocks, bf16."""
    t = _tileize(w, cols_slice).astype(BF16)
    return np.ascontiguousarray(t.reshape(P, -1))


def _prep_inputs(inputs):
    h = np.ascontiguousarray(np.asarray(inputs['hidden_states'], np.float32))
    hT = np.ascontiguousarray(h.T)
    cosT = np.ascontiguousarray(np.asarray(inputs['cos'], np.float32).T)
    sinT = np.ascontiguousarray(np.asarray(inputs['sin'], np.float32).T)
    q_a_w = np.asarray(inputs['q_a_w'], np.float32)
    q_b_w = np.asarray(inputs['q_b_w'], np.float32)
    kv_a_w = np.asarray(inputs['kv_a_w'], np.float32)
    kv_b_w = np.asarray(inputs['kv_b_w'], np.float32)
    o_w = np.asarray(inputs['o_w'], np.float32)
    gate_w = np.asarray(inputs['gate_w'], np.float32)
    up_w = np.asarray(inputs['up_w'], np.float32)
    down_w = np.asarray(inputs['down_w'], np.float32)

    pidx = np.arange(P)[:, None]
    cidx = np.arange(TCH)[None, :]
    masks = np.stack([(cidx - pidx >= P * k) for k in range(4)]
                     ).astype(np.float32).astype(F8)

    cosT2 = np.ascontiguousarray(np.vstack([cosT, cosT]))
    sinT2 = np.ascontiguousarray(np.vstack([sinT, sinT]))
    R = np.zeros((DR, DR), np.float32)
    R[np.arange(DR // 2), np.arange(DR // 2) + DR // 2] = -1.0
    R[np.arange(DR // 2) + DR // 2, np.arange(DR // 2)] = 1.0
    R2 = np.zeros((P, P), np.float32)
    R2[:DR, :DR] = R
    R2[DR:, DR:] = R
    rot2T = np.ascontiguousarray(R2.T)

    # hT tiles [P, NDT, S] -> chunk-major [P, NCH, NDT, TCH]
    hT_t = _tileize(hT)
    hT_cm = np.ascontiguousarray(
        hT_t.reshape(P, NDT, NCH, TCH).transpose(0, 2, 1, 3))
    hT8 = np.clip(hT_cm * SH, -240, 240).astype(F8)
    hTb = hT_cm.astype(BF16)

    def gup_flat(w, r):
        blocks = []
        for mcc in range(NMC):
            rows = min(P, IC - mcc * P)
            blocks.append(_flat_bf(w, np.s_[r * IC + mcc * P:
                                            r * IC + mcc * P + rows]))
        return np.ascontiguousarray(np.concatenate(blocks, axis=1))

    in_maps = []
    for r in range(NC):
        heads = range(r * HPC, (r + 1) * HPC)
        qb_cols = np.concatenate(
            [q_b_w[:, hh * (DN + DR):hh * (DN + DR) + DN] for hh in heads] +
            [q_b_w[:, hh * (DN + DR) + DN:(hh + 1) * (DN + DR)] for hh in heads],
            axis=1)
        kvb_cols = np.concatenate(
            [kv_b_w[:, hh * (DN + DV):hh * (DN + DV) + DN] for hh in heads] +
            [kv_b_w[:, hh * (DN + DV) + DN:(hh + 1) * (DN + DV)] for hh in heads],
            axis=1)
        qb_blocks = np.stack(
            [np.ascontiguousarray(
                _tileize(qb_cols, np.s_[mc * P:(mc + 1) * P])
                .reshape(P, -1)) for mc in range(NQB)])
        # o_w rows for own heads: [HPC*DV, D] -> [P, HPC, D]
        o_rows = np.concatenate(
            [o_w[hh * DV:(hh + 1) * DV, :] for hh in heads], axis=0)
        # down_w rows for own IC (padded to NMC*P): [NMC*P, D] ->
        # [P, NMC, D] tiles -> g-major [P, NDT*NMC*P]
        d_rows = np.zeros((NMC * P, D), np.float32)
        d_rows[:IC] = down_w[r * IC:(r + 1) * IC, :]
        d_t = _tileize(d_rows)                       # [P, NMC, D]
        d_gm = np.ascontiguousarray(
            d_t.reshape(P, NMC, NDT, P).transpose(0, 2, 1, 3)
            .reshape(P, -1)).astype(BF16)
        in_maps.append({
            'hT8': hT8,
            'hTb': hTb,
            'h_ownD': np.ascontiguousarray(hT[r * OC:(r + 1) * OC]),
            'qa_own': _q8(_tileize(q_a_w, np.s_[r * QAC:(r + 1) * QAC]), SW),
            'kva_own': _q8(np.concatenate([_tileize(kv_a_w, np.s_[r * KVAC:(r + 1) * KVAC]), np.zeros((P, NDT, KVAP - KVAC), np.float32)], axis=2), SW),
            'qb_own': _q8(qb_blocks.reshape(NQB * P, NQLT * P), SW),
            'kvb_own': _q8(_tileize(kvb_cols), SW),
            'o_own': _q8(_tileize(o_rows), SW),
            'gate_own': gup_flat(gate_w, r),
            'up_own': gup_flat(up_w, r),
            'down_own': d_gm,
            'cosT2': cosT2.astype(BF16),
            'sinT2': sinT2.astype(BF16),
            'rot2T': rot2T.astype(BF16),
            'masks': masks,
        })
    return in_maps


def kernel(**inputs) -> np.ndarray:
    if 'nc' not in _CACHE:
        _CACHE['nc'] = _build()
    nc = _CACHE['nc']
    from concourse.bass_utils import run_bass_kernel_spmd
    in_maps = _prep_inputs(inputs)
    res = run_bass_kernel_spmd(nc, in_maps, core_ids=list(range(NC)))
    outT = np.concatenate([res.results[r]['out'] for r in range(NC)], axis=0)
    return np.ascontiguousarray(outT.T)
